# revision 6
# baseline (speedup 1.0000x reference)
"""Trainium2 Bass kernel for nn_Cross_Attn_Token_to_Image (dense transformer cross-attention).

Sharding (8 cores, no collectives): core = (batch b in {0,1}) x (head-group g in {0..3},
2 heads each).  Host pre-transposes/casts activations to fp16 [E, tokens] slices and
pre-slices weights column-wise (q/cond/k1/k2/v) / row-wise (out_proj, pre-scaled by
0.5/16).  Each core computes a partial [2048, 1024] fp16 output (its heads' contribution
through its Wo row block); the host sums the 4 partials per batch in fp32 and folds the
linear bias terms (bv @ Wo + bo, exact because attention rows sum to 1).

Device dataflow per core (all matmuls fp16 operands, fp32 PSUM accumulation):
projections k1hT/k2hT [d,nk], vh [nk,d], qpT/cpT [d,nq] on the PE, with scores+exp
for the first query tiles interleaved into the k-projection stream.  Per attention
iteration (nq-tile x head): s = qhT^T k1hT on PE (N=512 into a [128,2048] fp32 PSUM
half); p = exp(s*scale) on ACT (FD=2048, accum_out row-sums r); softmax combine is a
single fused DVE op  pc = (p2 * r1/r2) + p1  (scalar_tensor_tensor); the per-query
1/r1 normalization rides the PE transpose for free via a scaled diagonal identity
(diag(16/r1), the 16x keeps fp16 away from subnormals and is folded into Wo on host);
uT = vh^T pcT accumulated on PE; final out = uT^T @ Wo emitted per tile once both
heads' uT exist.
"""
import numpy as np

NQ = 2048
NK = 4096
E = 1024
D = 128
SCALE = float(1.0 / np.sqrt(D))

_CACHED_NC = None


def _build():
    from contextlib import ExitStack

    import concourse.tile as tile
    from concourse import bacc, mybir
    from concourse.masks import make_identity

    F16 = mybir.dt.float16
    F32 = mybir.dt.float32
    AX = mybir.AxisListType
    ALU = mybir.AluOpType
    ACTF = mybir.ActivationFunctionType

    nc = bacc.Bacc("TRN2", target_bir_lowering=False, debug=False, num_devices=8)

    qT = nc.dram_tensor("qt", [E, NQ], F16, kind="ExternalInput").ap()
    cT = nc.dram_tensor("ct", [E, NQ], F16, kind="ExternalInput").ap()
    kT = nc.dram_tensor("kt", [E, NK], F16, kind="ExternalInput").ap()
    vT = nc.dram_tensor("vt", [E, NK], F16, kind="ExternalInput").ap()
    w_dram = {
        w: nc.dram_tensor(w, [E, 256], F16, kind="ExternalInput").ap()
        for w in ("wq", "wc", "wk1", "wk2", "wv")
    }
    wo = nc.dram_tensor("wo", [256, E], F16, kind="ExternalInput").ap()
    b_dram = {
        b: nc.dram_tensor(b, [128, 2], F32, kind="ExternalInput").ap()
        for b in ("bq", "bc", "bk1", "bk2")
    }
    outp = nc.dram_tensor("outp", [NQ, E], F16, kind="ExternalOutput").ap()

    with tile.TileContext(nc) as tc, ExitStack() as ctx:
        const = ctx.enter_context(tc.tile_pool(name="const", bufs=1))
        persist = ctx.enter_context(tc.tile_pool(name="persist", bufs=1))

        b_sb = {}

        def _load_b(b):
            b_sb[b] = const.tile([128, 2], F32, name=b)
            nc.sync.dma_start(out=b_sb[b], in_=b_dram[b])

        w_sb = {}

        def _load_w(w):
            w_sb[w] = const.tile([128, 8, 256], F16, name=w)
            nc.sync.dma_start(
                out=w_sb[w], in_=w_dram[w].rearrange("(c p) n -> p c n", p=128)
            )

        # double-buffered by head: projections for head h land in slot h
        qpT = persist.tile([128, 2, NQ], F16, name="qpT")
        cpT = persist.tile([128, 2, NQ], F16, name="cpT")
        k1hT = persist.tile([128, 2, NK], F16, name="k1hT")
        k2hT = persist.tile([128, 2, NK], F16, name="k2hT")
        vh = persist.tile([128, 32, 2, 128], F16, name="vh")
        u_store = persist.tile([128, 2, 16, 128], F16, name="u_store")
        # 1/r per (head, tile, m): rinv_store[:, h, t, 0] = 1/r1 feeds the
        # deferred out-proj normalization; [..., 1] = 1/r2 feeds the combine.
        rinv_store = persist.tile([128, 2, 16, 2], F32, name="rinv_store")

        acts = ctx.enter_context(tc.tile_pool(name="acts", bufs=2))
        ppsum = ctx.enter_context(tc.tile_pool(name="ppsum", bufs=2, space="PSUM"))
        spsum = ctx.enter_context(tc.tile_pool(name="spsum", bufs=1, space="PSUM"))
        ptpsum = ctx.enter_context(tc.tile_pool(name="ptpsum", bufs=1, space="PSUM"))
        upsum = ctx.enter_context(tc.tile_pool(name="upsum", bufs=1, space="PSUM"))
        ppool = ctx.enter_context(tc.tile_pool(name="ppool", bufs=4))
        ptpool = ctx.enter_context(tc.tile_pool(name="ptpool", bufs=3))
        small = ctx.enter_context(tc.tile_pool(name="small", bufs=4))
        opool = ctx.enter_context(tc.tile_pool(name="opool", bufs=3))

        def proj_one(src_, wname, bname, dst, blk):
            a_sb = acts.tile([128, 8, 512], F16, tag="act_in")
            nc.sync.dma_start(
                out=a_sb,
                in_=src_.rearrange("(c p) n -> p c n", p=128)[
                    :, :, blk * 512 : (blk + 1) * 512
                ],
            )
            for h in range(2):
                hs = slice(h * 128, (h + 1) * 128)
                ps = ppsum.tile([128, 512], F32, tag="proj")
                for e in range(8):
                    nc.tensor.matmul(
                        ps,
                        lhsT=w_sb[wname][:, e, hs],
                        rhs=a_sb[:, e, :],
                        start=(e == 0),
                        stop=(e == 7),
                    )
                nc.vector.tensor_scalar_add(
                    dst[:, h, blk * 512 : (blk + 1) * 512],
                    ps,
                    b_sb[bname][:, h : h + 1],
                )

        def proj_qc_block(blk):
            proj_one(qT, "wq", "bq", qpT, blk)
            proj_one(cT, "wc", "bc", cpT, blk)

        def proj_k_block(blk):
            a_sb = acts.tile([128, 8, 512], F16, tag="act_in")
            nc.sync.dma_start(
                out=a_sb,
                in_=kT.rearrange("(c p) n -> p c n", p=128)[
                    :, :, blk * 512 : (blk + 1) * 512
                ],
            )
            for wname, bname, dst in (("wk1", "bk1", k1hT), ("wk2", "bk2", k2hT)):
                for h in range(2):
                    hs = slice(h * 128, (h + 1) * 128)
                    ps = ppsum.tile([128, 512], F32, tag="proj")
                    for e in range(8):
                        nc.tensor.matmul(
                            ps,
                            lhsT=w_sb[wname][:, e, hs],
                            rhs=a_sb[:, e, :],
                            start=(e == 0),
                            stop=(e == 7),
                        )
                    nc.vector.tensor_scalar_add(
                        dst[:, h, blk * 512 : (blk + 1) * 512],
                        ps,
                        b_sb[bname][:, h : h + 1],
                    )

        def proj_v_block(blk):
            a_sb = acts.tile([128, 8, 512], F16, tag="act_in")
            nc.sync.dma_start(
                out=a_sb,
                in_=vT.rearrange("(c p) n -> p c n", p=128)[
                    :, :, blk * 512 : (blk + 1) * 512
                ],
            )
            for tp in range(2):
                ps = ppsum.tile([128, 512], F32, tag="proj", name="vps")
                for half in range(2):
                    tt = tp * 2 + half
                    for e in range(8):
                        nc.tensor.matmul(
                            ps[:, half * 256 : (half + 1) * 256],
                            lhsT=a_sb[:, e, tt * 128 : (tt + 1) * 128],
                            rhs=w_sb["wv"][:, e, :],
                            start=(e == 0),
                            stop=(e == 7),
                        )
                nc.vector.tensor_copy(
                    out=vh[:, blk * 4 + tp * 2 : blk * 4 + tp * 2 + 2, :, :], in_=ps
                )

        def attn_scores_half(t, h, m, half, state):
            """Scores+exp for one nk-half of one score matrix m of tile t."""
            src_, khT = (qpT, k1hT) if m == 1 else (cpT, k2hT)
            if half == 0:
                if "rp" not in state:
                    state["rp"] = small.tile(
                        [128, 2, 2], F32, tag="rp", name=f"rp_{t}_{h}"
                    )
                state[m] = ppool.tile(
                    [128, NK], F16, tag=f"p{m}", name=f"p{m}_{t}_{h}"
                )
            p_sb = state[m]
            rp_m = state["rp"][:, m - 1, :]
            lhsT = src_[:, h, t * 128 : (t + 1) * 128]
            ps = spsum.tile([128, 2048], F32, tag="s", name=f"s{m}_{t}_{h}_{half}")
            for j in range(4):
                nc.tensor.matmul(
                    ps[:, j * 512 : (j + 1) * 512],
                    lhsT=lhsT,
                    rhs=khT[:, h, half * 2048 + j * 512 : half * 2048 + (j + 1) * 512],
                    start=True,
                    stop=True,
                )
            nc.scalar.activation(
                out=p_sb[:, half * 2048 : (half + 1) * 2048],
                in_=ps,
                func=ACTF.Exp,
                scale=SCALE,
                accum_out=rp_m[:, half : half + 1],
            )

        def attn_tail(t, h, state):
            """Fused combine, transpose, attn@v; final projection after h=1.

            p is left unnormalized: pc = p1 + (r1/r2) p2 = 2 r1 * attn.  The
            1/r1 factor is applied per-query at the out-proj (queries land on
            partitions there), with 0.5 folded into wo host-side.
            """
            pc = state[1]
            p2 = state[2]
            r = small.tile([128, 2], F32, tag="r2")
            nc.vector.tensor_reduce(out=r, in_=state["rp"], axis=AX.X, op=ALU.add)
            rinv = rinv_store[:, h, t, :]
            nc.vector.reciprocal(out=rinv, in_=r)
            # c21 = r1/r2: relative weight of the second softmax in the combine
            c21 = small.tile([128, 1], F32, tag="c21")
            nc.vector.tensor_mul(out=c21, in0=r[:, 0:1], in1=rinv[:, 1:2])
            for half in range(2):
                sl = slice(half * 2048, (half + 1) * 2048)
                nc.vector.scalar_tensor_tensor(
                    out=pc[:, sl],
                    in0=p2[:, sl],
                    scalar=c21,
                    in1=pc[:, sl],
                    op0=ALU.mult,
                    op1=ALU.add,
                )
            u_ps = upsum.tile([128, 128], F32, tag="u")
            for qt in range(4):
                pt_ps = ptpsum.tile([128, 1024], F16, tag="pt")
                for c8 in range(8):
                    ck = qt * 8 + c8
                    nc.tensor.transpose(
                        pt_ps[:, c8 * 128 : (c8 + 1) * 128],
                        in_=pc[:, ck * 128 : (ck + 1) * 128],
                        identity=identity16,
                    )
                pt_sb = ptpool.tile([128, 1024], F16, tag="pt_sb")
                nc.vector.tensor_copy(out=pt_sb, in_=pt_ps)
                for c8 in range(8):
                    ck = qt * 8 + c8
                    nc.tensor.matmul(
                        u_ps,
                        lhsT=vh[:, ck, h, :],
                        rhs=pt_sb[:, c8 * 128 : (c8 + 1) * 128],
                        start=(ck == 0),
                        stop=(ck == 31),
                    )
            nc.vector.tensor_copy(out=u_store[:, h, t, :], in_=u_ps)

            if h == 1:
                o_sb = opool.tile([128, E], F16, tag="o_sb")
                for j in range(2):
                    o_ps0 = ppsum.tile([128, 512], F32, tag="proj", name="ops0")
                    nc.tensor.matmul(
                        o_ps0,
                        lhsT=u_store[:, 0, t, :],
                        rhs=wo_sb[:, 0, j * 512 : (j + 1) * 512],
                        start=True,
                        stop=True,
                    )
                    o_ps1 = ppsum.tile([128, 512], F32, tag="proj", name="ops1")
                    nc.tensor.matmul(
                        o_ps1,
                        lhsT=u_store[:, 1, t, :],
                        rhs=wo_sb[:, 1, j * 512 : (j + 1) * 512],
                        start=True,
                        stop=True,
                    )
                    otmp = opool.tile([128, 512], F16, tag="otmp")
                    nc.vector.tensor_scalar_mul(
                        otmp, o_ps1, rinv_store[:, 1, t, 0:1]
                    )
                    nc.vector.scalar_tensor_tensor(
                        out=o_sb[:, j * 512 : (j + 1) * 512],
                        in0=o_ps0,
                        scalar=rinv_store[:, 0, t, 0:1],
                        in1=otmp,
                        op0=ALU.mult,
                        op1=ALU.add,
                    )
                nc.sync.dma_start(out=outp[t * 128 : (t + 1) * 128, :], in_=o_sb)

        def attention_tile(t, h):
            state = {}
            for m in (1, 2):
                for half in range(2):
                    attn_scores_half(t, h, m, half, state)
            attn_tail(t, h, state)

        # interleaved schedule: q/c block 0 first (DMA-order: wq+bq before the q
        # activations, wc+bc next, k weights after), then k blocks with streamed
        # scores+exp for tiles 0..3 of head 0, then v, remaining q/c, tails.
        _load_w("wq")
        _load_b("bq")
        proj_one(qT, "wq", "bq", qpT, 0)
        _load_w("wc")
        _load_b("bc")
        proj_one(cT, "wc", "bc", cpT, 0)
        _load_w("wk1")
        _load_w("wk2")
        _load_b("bk1")
        _load_b("bk2")
        early = {t: {} for t in range(4)}
        for blk in range(8):
            proj_k_block(blk)
            if blk % 4 == 3:
                half = blk // 4
                for t in range(4):
                    for m in (1, 2):
                        attn_scores_half(t, 0, m, half, early[t])
        _load_w("wv")
        identity16 = const.tile([128, 128], F16)
        make_identity(nc, identity16)
        wo_sb = const.tile([128, 2, E], F16)
        nc.sync.dma_start(out=wo_sb, in_=wo.rearrange("(c p) n -> p c n", p=128))
        for blk in range(8):
            proj_v_block(blk)
        attn_tail(0, 0, early[0])
        sts = {}
        proj_qc_block(1)
        for nxt, tl in ((4, 1), (5, 2)):
            sts[nxt] = {}
            for m in (1, 2):
                for half in range(2):
                    attn_scores_half(nxt, 0, m, half, sts[nxt])
            attn_tail(tl, 0, early[tl])
        proj_qc_block(2)
        sts[6] = {}
        for m in (1, 2):
            for half in range(2):
                attn_scores_half(6, 0, m, half, sts[6])
        attn_tail(3, 0, early[3])
        proj_qc_block(3)
        sts[7] = {}
        for m in (1, 2):
            for half in range(2):
                attn_scores_half(7, 0, m, half, sts[7])
        for t in range(4, 8):
            attn_tail(t, 0, sts[t])
        for t in range(8, 16):
            attention_tile(t, 0)
        for t in range(16):
            attention_tile(t, 1)

    nc.compile()
    return nc


def _get_nc():
    global _CACHED_NC
    if _CACHED_NC is None:
        _CACHED_NC = _build()
    return _CACHED_NC


def make_in_maps(q, k, v, cond_feat, Wq, Wc, Wk2, Wv, Wo, bq, bc, bk2):
    f16 = lambda x: np.ascontiguousarray(x, dtype=np.float16)
    b2 = lambda x: np.ascontiguousarray(x.reshape(2, 128).T, dtype=np.float32)

    in_maps = []
    for core in range(8):
        b, g = core // 4, core % 4
        sl = slice(g * 256, (g + 1) * 256)
        sl2 = slice(E + g * 256, E + (g + 1) * 256)
        in_maps.append(
            {
                "qt": f16(q[b].T),
                "ct": f16(cond_feat[b].T),
                "kt": f16(k[b].T),
                "vt": f16(v[b].T),
                "wq": f16(Wq[:, sl]),
                "wc": f16(Wc[:, sl]),
                "wk1": f16(Wk2[:, sl]),
                "wk2": f16(Wk2[:, sl2]),
                "wv": f16(Wv[:, sl]),
                "wo": f16(Wo[sl, :] * 0.5),
                "bq": b2(bq[sl]),
                "bc": b2(bc[sl]),
                "bk1": b2(bk2[sl]),
                "bk2": b2(bk2[sl2]),
            }
        )
    return in_maps


def kernel(q, k, v, cond_feat, Wq, bq, Wc, bc, Wk2, bk2, Wv, bv, Wo, bo):
    from concourse.bass_utils import run_bass_kernel_spmd

    q = np.asarray(q, np.float32)
    k = np.asarray(k, np.float32)
    v = np.asarray(v, np.float32)
    cond_feat = np.asarray(cond_feat, np.float32)
    Wq, bq = np.asarray(Wq, np.float32), np.asarray(bq, np.float32)
    Wc, bc = np.asarray(Wc, np.float32), np.asarray(bc, np.float32)
    Wk2, bk2 = np.asarray(Wk2, np.float32), np.asarray(bk2, np.float32)
    Wv, bv = np.asarray(Wv, np.float32), np.asarray(bv, np.float32)
    Wo, bo = np.asarray(Wo, np.float32), np.asarray(bo, np.float32)

    in_maps = make_in_maps(q, k, v, cond_feat, Wq, Wc, Wk2, Wv, Wo, bq, bc, bk2)

    nc = _get_nc()
    res = run_bass_kernel_spmd(nc, in_maps, core_ids=list(range(8)))

    out = np.zeros((2, NQ, E), np.float32)
    for core in range(8):
        out[core // 4] += res.results[core]["outp"].astype(np.float32)
    out += (bv @ Wo + bo)[None, None, :]
    return out


# revision 9
# speedup vs baseline: 1.2898x; 1.2898x over previous
"""Trainium2 Bass kernel for nn_Cross_Attn_Token_to_Image (dense transformer cross-attention).

Sharding (8 cores, no collectives): core = (batch b in {0,1}) x (head-group g in {0..3},
2 heads each).  Host pre-transposes/casts activations to fp16 [E, tokens] slices and
pre-slices weights column-wise (q/cond/k1/k2/v) / row-wise (out_proj, pre-scaled by
0.5/16).  Each core computes a partial [2048, 1024] fp16 output (its heads' contribution
through its Wo row block); the host sums the 4 partials per batch in fp32 and folds the
linear bias terms (bv @ Wo + bo, exact because attention rows sum to 1).

Device dataflow per core (all matmuls fp16 operands, fp32 PSUM accumulation):
projections k1hT/k2hT [d,nk], vh [nk,d], qpT/cpT [d,nq] on the PE, with scores+exp
for the first query tiles interleaved into the k-projection stream.  Per attention
iteration (nq-tile x head): s = qhT^T k1hT on PE (N=512 into a [128,2048] fp32 PSUM
half); p = exp(s*scale) on ACT (FD=2048, accum_out row-sums r); softmax combine is a
single fused DVE op  pc = (p2 * r1/r2) + p1  (scalar_tensor_tensor); the per-query
1/r1 normalization rides the PE transpose for free via a scaled diagonal identity
(diag(16/r1), the 16x keeps fp16 away from subnormals and is folded into Wo on host);
uT = vh^T pcT accumulated on PE; final out = uT^T @ Wo emitted per tile once both
heads' uT exist.
"""
import numpy as np

NQ = 2048
NK = 4096
E = 1024
D = 128
SCALE = float(1.0 / np.sqrt(D))

_CACHED_NC = None


def _build():
    from contextlib import ExitStack

    import concourse.tile as tile
    from concourse import bacc, mybir
    from concourse.masks import make_identity

    F16 = mybir.dt.float16
    F32 = mybir.dt.float32
    AX = mybir.AxisListType
    ALU = mybir.AluOpType
    ACTF = mybir.ActivationFunctionType

    nc = bacc.Bacc("TRN2", target_bir_lowering=False, debug=False, num_devices=8)

    qT = nc.dram_tensor("qt", [E, NQ], F16, kind="ExternalInput").ap()
    cT = nc.dram_tensor("ct", [E, NQ], F16, kind="ExternalInput").ap()
    kT = nc.dram_tensor("kt", [E, NK], F16, kind="ExternalInput").ap()
    vT = nc.dram_tensor("vt", [E, NK], F16, kind="ExternalInput").ap()
    w_dram = {
        w: nc.dram_tensor(w, [E, 256], F16, kind="ExternalInput").ap()
        for w in ("wq", "wc", "wk1", "wk2", "wv")
    }
    wo = nc.dram_tensor("wo", [256, E], F16, kind="ExternalInput").ap()
    b_dram = {
        b: nc.dram_tensor(b, [128, 2], F32, kind="ExternalInput").ap()
        for b in ("bq", "bc", "bk1", "bk2")
    }
    outp = nc.dram_tensor("outp", [NQ, E], F16, kind="ExternalOutput").ap()

    with tile.TileContext(nc) as tc, ExitStack() as ctx:
        const = ctx.enter_context(tc.tile_pool(name="const", bufs=1))
        persist = ctx.enter_context(tc.tile_pool(name="persist", bufs=1))

        b_sb = {}

        def _load_b(b):
            b_sb[b] = const.tile([128, 2], F32, name=b)
            nc.sync.dma_start(out=b_sb[b], in_=b_dram[b])

        w_sb = {}

        def _load_w(w):
            w_sb[w] = const.tile([128, 8, 256], F16, name=w)
            nc.sync.dma_start(
                out=w_sb[w], in_=w_dram[w].rearrange("(c p) n -> p c n", p=128)
            )

        # double-buffered by head: projections for head h land in slot h
        qpT = persist.tile([128, 2, NQ], F16, name="qpT")
        cpT = persist.tile([128, 2, NQ], F16, name="cpT")
        k1hT = persist.tile([128, 2, NK], F16, name="k1hT")
        k2hT = persist.tile([128, 2, NK], F16, name="k2hT")
        vh = persist.tile([128, 32, 2, 128], F16, name="vh")
        u_store = persist.tile([128, 2, 16, 128], F16, name="u_store")
        # 1/r per (head, tile, m): rinv_store[:, h, t, 0] = 1/r1 feeds the
        # deferred out-proj normalization; [..., 1] = 1/r2 feeds the combine.
        rinv_store = persist.tile([128, 2, 16, 2], F32, name="rinv_store")

        acts = ctx.enter_context(tc.tile_pool(name="acts", bufs=2))
        ppsum = ctx.enter_context(tc.tile_pool(name="ppsum", bufs=2, space="PSUM"))
        spsum = ctx.enter_context(tc.tile_pool(name="spsum", bufs=2, space="PSUM"))
        ptpsum = ctx.enter_context(tc.tile_pool(name="ptpsum", bufs=1, space="PSUM"))
        upsum = ctx.enter_context(tc.tile_pool(name="upsum", bufs=1, space="PSUM"))
        ppool = ctx.enter_context(tc.tile_pool(name="ppool", bufs=4))
        ptpool = ctx.enter_context(tc.tile_pool(name="ptpool", bufs=3))
        small = ctx.enter_context(tc.tile_pool(name="small", bufs=4))
        opool = ctx.enter_context(tc.tile_pool(name="opool", bufs=3))

        def proj_one(src_, wname, bname, dst, blk):
            a_sb = acts.tile([128, 8, 512], F16, tag="act_in")
            nc.sync.dma_start(
                out=a_sb,
                in_=src_.rearrange("(c p) n -> p c n", p=128)[
                    :, :, blk * 512 : (blk + 1) * 512
                ],
            )
            for h in range(2):
                hs = slice(h * 128, (h + 1) * 128)
                ps = ppsum.tile([128, 512], F32, tag="proj")
                for e in range(8):
                    nc.tensor.matmul(
                        ps,
                        lhsT=w_sb[wname][:, e, hs],
                        rhs=a_sb[:, e, :],
                        start=(e == 0),
                        stop=(e == 7),
                    )
                nc.vector.tensor_scalar_add(
                    dst[:, h, blk * 512 : (blk + 1) * 512],
                    ps,
                    b_sb[bname][:, h : h + 1],
                )

        def proj_qc_block(blk):
            proj_one(qT, "wq", "bq", qpT, blk)
            proj_one(cT, "wc", "bc", cpT, blk)

        def proj_k_block(blk):
            a_sb = acts.tile([128, 8, 512], F16, tag="act_in")
            nc.sync.dma_start(
                out=a_sb,
                in_=kT.rearrange("(c p) n -> p c n", p=128)[
                    :, :, blk * 512 : (blk + 1) * 512
                ],
            )
            for wname, bname, dst in (("wk1", "bk1", k1hT), ("wk2", "bk2", k2hT)):
                for h in range(2):
                    hs = slice(h * 128, (h + 1) * 128)
                    ps = ppsum.tile([128, 512], F32, tag="proj")
                    for e in range(8):
                        nc.tensor.matmul(
                            ps,
                            lhsT=w_sb[wname][:, e, hs],
                            rhs=a_sb[:, e, :],
                            start=(e == 0),
                            stop=(e == 7),
                        )
                    nc.vector.tensor_scalar_add(
                        dst[:, h, blk * 512 : (blk + 1) * 512],
                        ps,
                        b_sb[bname][:, h : h + 1],
                    )

        def proj_v_block(blk):
            a_sb = acts.tile([128, 8, 512], F16, tag="act_in")
            nc.sync.dma_start(
                out=a_sb,
                in_=vT.rearrange("(c p) n -> p c n", p=128)[
                    :, :, blk * 512 : (blk + 1) * 512
                ],
            )
            for tp in range(2):
                ps = ppsum.tile([128, 512], F32, tag="proj", name="vps")
                for half in range(2):
                    tt = tp * 2 + half
                    for e in range(8):
                        nc.tensor.matmul(
                            ps[:, half * 256 : (half + 1) * 256],
                            lhsT=a_sb[:, e, tt * 128 : (tt + 1) * 128],
                            rhs=w_sb["wv"][:, e, :],
                            start=(e == 0),
                            stop=(e == 7),
                        )
                nc.vector.tensor_copy(
                    out=vh[:, blk * 4 + tp * 2 : blk * 4 + tp * 2 + 2, :, :], in_=ps
                )

        def attn_scores_q(t, h, m, qt, state):
            """Scores+exp for one quarter of one score matrix m of tile t."""
            src_, khT = (qpT, k1hT) if m == 1 else (cpT, k2hT)
            if qt == 0:
                if "rp" not in state:
                    state["rp"] = small.tile(
                        [128, 2, 4], F32, tag="rp", name=f"rp_{t}_{h}"
                    )
                state[m] = ppool.tile(
                    [128, NK], F16, tag=f"p{m}", name=f"p{m}_{t}_{h}"
                )
            p_sb = state[m]
            rp_m = state["rp"][:, m - 1, :]
            lhsT = src_[:, h, t * 128 : (t + 1) * 128]
            ps = spsum.tile([128, 1024], F32, tag="s", name=f"s{m}_{t}_{h}_{qt}")
            for j in range(2):
                nc.tensor.matmul(
                    ps[:, j * 512 : (j + 1) * 512],
                    lhsT=lhsT,
                    rhs=khT[:, h, qt * 1024 + j * 512 : qt * 1024 + (j + 1) * 512],
                    start=True,
                    stop=True,
                )
            nc.scalar.activation(
                out=p_sb[:, qt * 1024 : (qt + 1) * 1024],
                in_=ps,
                func=ACTF.Exp,
                scale=SCALE,
                accum_out=rp_m[:, qt : qt + 1],
            )

        def attn_tail(t, h, state):
            """Fused combine, transpose, attn@v; final projection after h=1.

            p is left unnormalized: pc = p1 + (r1/r2) p2 = 2 r1 * attn.  The
            1/r1 factor is applied per-query at the out-proj (queries land on
            partitions there), with 0.5 folded into wo host-side.
            """
            pc = state[1]
            p2 = state[2]
            r = small.tile([128, 2], F32, tag="r2")
            nc.vector.tensor_reduce(out=r, in_=state["rp"], axis=AX.X, op=ALU.add)
            rinv = rinv_store[:, h, t, :]
            nc.vector.reciprocal(out=rinv, in_=r)
            # c21 = r1/r2: relative weight of the second softmax in the combine
            c21 = small.tile([128, 1], F32, tag="c21")
            nc.vector.tensor_mul(out=c21, in0=r[:, 0:1], in1=rinv[:, 1:2])
            for half in range(2):
                sl = slice(half * 2048, (half + 1) * 2048)
                nc.vector.scalar_tensor_tensor(
                    out=pc[:, sl],
                    in0=p2[:, sl],
                    scalar=c21,
                    in1=pc[:, sl],
                    op0=ALU.mult,
                    op1=ALU.add,
                )
            u_ps = upsum.tile([128, 128], F32, tag="u")
            for qt in range(4):
                pt_ps = ptpsum.tile([128, 1024], F16, tag="pt")
                for c8 in range(8):
                    ck = qt * 8 + c8
                    nc.tensor.transpose(
                        pt_ps[:, c8 * 128 : (c8 + 1) * 128],
                        in_=pc[:, ck * 128 : (ck + 1) * 128],
                        identity=identity16,
                    )
                pt_sb = ptpool.tile([128, 1024], F16, tag="pt_sb")
                nc.vector.tensor_copy(out=pt_sb, in_=pt_ps)
                for c8 in range(8):
                    ck = qt * 8 + c8
                    nc.tensor.matmul(
                        u_ps,
                        lhsT=vh[:, ck, h, :],
                        rhs=pt_sb[:, c8 * 128 : (c8 + 1) * 128],
                        start=(ck == 0),
                        stop=(ck == 31),
                    )
            nc.vector.tensor_copy(out=u_store[:, h, t, :], in_=u_ps)

            if h == 1:
                o_sb = opool.tile([128, E], F16, tag="o_sb")
                for j in range(2):
                    o_ps0 = ppsum.tile([128, 512], F32, tag="proj", name="ops0")
                    nc.tensor.matmul(
                        o_ps0,
                        lhsT=u_store[:, 0, t, :],
                        rhs=wo_sb[:, 0, j * 512 : (j + 1) * 512],
                        start=True,
                        stop=True,
                    )
                    o_ps1 = ppsum.tile([128, 512], F32, tag="proj", name="ops1")
                    nc.tensor.matmul(
                        o_ps1,
                        lhsT=u_store[:, 1, t, :],
                        rhs=wo_sb[:, 1, j * 512 : (j + 1) * 512],
                        start=True,
                        stop=True,
                    )
                    otmp = opool.tile([128, 512], F16, tag="otmp")
                    nc.vector.tensor_scalar_mul(
                        otmp, o_ps1, rinv_store[:, 1, t, 0:1]
                    )
                    nc.vector.scalar_tensor_tensor(
                        out=o_sb[:, j * 512 : (j + 1) * 512],
                        in0=o_ps0,
                        scalar=rinv_store[:, 0, t, 0:1],
                        in1=otmp,
                        op0=ALU.mult,
                        op1=ALU.add,
                    )
                nc.sync.dma_start(out=outp[t * 128 : (t + 1) * 128, :], in_=o_sb)

        def attention_tile(t, h):
            state = {}
            for m in (1, 2):
                for qt in range(4):
                    attn_scores_q(t, h, m, qt, state)
            attn_tail(t, h, state)

        # interleaved schedule: q/c block 0 first (DMA-order: wq+bq before the q
        # activations, wc+bc next, k weights after), then k blocks with streamed
        # scores+exp for tiles 0..3 of head 0, then v, remaining q/c, tails.
        _load_w("wq")
        _load_b("bq")
        proj_one(qT, "wq", "bq", qpT, 0)
        _load_w("wc")
        _load_b("bc")
        proj_one(cT, "wc", "bc", cpT, 0)
        _load_w("wk1")
        _load_w("wk2")
        _load_b("bk1")
        _load_b("bk2")
        early = {t: {} for t in range(4)}
        for blk in range(8):
            proj_k_block(blk)
            if blk % 2 == 1:
                qt = blk // 2
                for t in range(4):
                    for m in (1, 2):
                        attn_scores_q(t, 0, m, qt, early[t])
        _load_w("wv")
        identity16 = const.tile([128, 128], F16)
        make_identity(nc, identity16)
        wo_sb = const.tile([128, 2, E], F16)
        nc.sync.dma_start(out=wo_sb, in_=wo.rearrange("(c p) n -> p c n", p=128))
        for blk in range(8):
            proj_v_block(blk)
        attn_tail(0, 0, early[0])
        sts = {}
        proj_qc_block(1)
        for nxt, tl in ((4, 1), (5, 2)):
            sts[nxt] = {}
            for m in (1, 2):
                for qt in range(4):
                    attn_scores_q(nxt, 0, m, qt, sts[nxt])
            attn_tail(tl, 0, early[tl])
        proj_qc_block(2)
        sts[6] = {}
        for m in (1, 2):
            for qt in range(4):
                attn_scores_q(6, 0, m, qt, sts[6])
        attn_tail(3, 0, early[3])
        proj_qc_block(3)
        sts[7] = {}
        for m in (1, 2):
            for qt in range(4):
                attn_scores_q(7, 0, m, qt, sts[7])
        for t in range(4, 8):
            attn_tail(t, 0, sts[t])
        for t in range(8, 16):
            attention_tile(t, 0)
        for t in range(16):
            attention_tile(t, 1)

    nc.compile()
    return nc


def _get_nc():
    global _CACHED_NC
    if _CACHED_NC is None:
        _CACHED_NC = _build()
    return _CACHED_NC


def make_in_maps(q, k, v, cond_feat, Wq, Wc, Wk2, Wv, Wo, bq, bc, bk2):
    f16 = lambda x: np.ascontiguousarray(x, dtype=np.float16)
    b2 = lambda x: np.ascontiguousarray(x.reshape(2, 128).T, dtype=np.float32)

    in_maps = []
    for core in range(8):
        b, g = core // 4, core % 4
        sl = slice(g * 256, (g + 1) * 256)
        sl2 = slice(E + g * 256, E + (g + 1) * 256)
        in_maps.append(
            {
                "qt": f16(q[b].T),
                "ct": f16(cond_feat[b].T),
                "kt": f16(k[b].T),
                "vt": f16(v[b].T),
                "wq": f16(Wq[:, sl]),
                "wc": f16(Wc[:, sl]),
                "wk1": f16(Wk2[:, sl]),
                "wk2": f16(Wk2[:, sl2]),
                "wv": f16(Wv[:, sl]),
                "wo": f16(Wo[sl, :] * 0.5),
                "bq": b2(bq[sl]),
                "bc": b2(bc[sl]),
                "bk1": b2(bk2[sl]),
                "bk2": b2(bk2[sl2]),
            }
        )
    return in_maps


def kernel(q, k, v, cond_feat, Wq, bq, Wc, bc, Wk2, bk2, Wv, bv, Wo, bo):
    from concourse.bass_utils import run_bass_kernel_spmd

    q = np.asarray(q, np.float32)
    k = np.asarray(k, np.float32)
    v = np.asarray(v, np.float32)
    cond_feat = np.asarray(cond_feat, np.float32)
    Wq, bq = np.asarray(Wq, np.float32), np.asarray(bq, np.float32)
    Wc, bc = np.asarray(Wc, np.float32), np.asarray(bc, np.float32)
    Wk2, bk2 = np.asarray(Wk2, np.float32), np.asarray(bk2, np.float32)
    Wv, bv = np.asarray(Wv, np.float32), np.asarray(bv, np.float32)
    Wo, bo = np.asarray(Wo, np.float32), np.asarray(bo, np.float32)

    in_maps = make_in_maps(q, k, v, cond_feat, Wq, Wc, Wk2, Wv, Wo, bq, bc, bk2)

    nc = _get_nc()
    res = run_bass_kernel_spmd(nc, in_maps, core_ids=list(range(8)))

    out = np.zeros((2, NQ, E), np.float32)
    for core in range(8):
        out[core // 4] += res.results[core]["outp"].astype(np.float32)
    out += (bv @ Wo + bo)[None, None, :]
    return out


# revision 12
# speedup vs baseline: 1.2971x; 1.0057x over previous
"""Trainium2 Bass kernel for nn_Cross_Attn_Token_to_Image (dense transformer cross-attention).

Sharding (8 cores, no collectives): core = (batch b in {0,1}) x (head-group g in {0..3},
2 heads each).  Host pre-transposes/casts activations to fp16 [E, tokens] slices and
pre-slices weights column-wise (q/cond/k1/k2/v) / row-wise (out_proj, pre-scaled by
0.5/16).  Each core computes a partial [2048, 1024] fp16 output (its heads' contribution
through its Wo row block); the host sums the 4 partials per batch in fp32 and folds the
linear bias terms (bv @ Wo + bo, exact because attention rows sum to 1).

Device dataflow per core (all matmuls fp16 operands, fp32 PSUM accumulation):
projections k1hT/k2hT [d,nk], vh [nk,d], qpT/cpT [d,nq] on the PE, with scores+exp
for the first query tiles interleaved into the k-projection stream.  Per attention
iteration (nq-tile x head): s = qhT^T k1hT on PE (N=512 into a [128,2048] fp32 PSUM
half); p = exp(s*scale) on ACT (FD=2048, accum_out row-sums r); softmax combine is a
single fused DVE op  pc = (p2 * r1/r2) + p1  (scalar_tensor_tensor); the per-query
1/r1 normalization rides the PE transpose for free via a scaled diagonal identity
(diag(16/r1), the 16x keeps fp16 away from subnormals and is folded into Wo on host);
uT = vh^T pcT accumulated on PE; final out = uT^T @ Wo emitted per tile once both
heads' uT exist.
"""
import numpy as np

NQ = 2048
NK = 4096
E = 1024
D = 128
SCALE = float(1.0 / np.sqrt(D))

_CACHED_NC = None


def _build():
    from contextlib import ExitStack

    import concourse.tile as tile
    from concourse import bacc, mybir
    from concourse.masks import make_identity

    F16 = mybir.dt.float16
    F32 = mybir.dt.float32
    AX = mybir.AxisListType
    ALU = mybir.AluOpType
    ACTF = mybir.ActivationFunctionType

    nc = bacc.Bacc("TRN2", target_bir_lowering=False, debug=False, num_devices=8)

    qT = nc.dram_tensor("qt", [E, NQ], F16, kind="ExternalInput").ap()
    cT = nc.dram_tensor("ct", [E, NQ], F16, kind="ExternalInput").ap()
    kT = nc.dram_tensor("kt", [E, NK], F16, kind="ExternalInput").ap()
    vT = nc.dram_tensor("vt", [E, NK], F16, kind="ExternalInput").ap()
    w_dram = {
        w: nc.dram_tensor(w, [E, 256], F16, kind="ExternalInput").ap()
        for w in ("wq", "wc", "wk1", "wk2", "wv")
    }
    wo = nc.dram_tensor("wo", [256, E], F16, kind="ExternalInput").ap()
    b_dram = {
        b: nc.dram_tensor(b, [128, 2], F32, kind="ExternalInput").ap()
        for b in ("bq", "bc", "bk1", "bk2")
    }
    outp = nc.dram_tensor("outp", [NQ, E], F16, kind="ExternalOutput").ap()

    with tile.TileContext(nc) as tc, ExitStack() as ctx:
        const = ctx.enter_context(tc.tile_pool(name="const", bufs=1))
        persist = ctx.enter_context(tc.tile_pool(name="persist", bufs=1))

        b_sb = {}

        def _load_b(b):
            b_sb[b] = const.tile([128, 2], F32, name=b)
            nc.sync.dma_start(out=b_sb[b], in_=b_dram[b])

        w_sb = {}

        def _load_w(w):
            w_sb[w] = const.tile([128, 8, 256], F16, name=w)
            nc.sync.dma_start(
                out=w_sb[w], in_=w_dram[w].rearrange("(c p) n -> p c n", p=128)
            )

        # double-buffered by head: projections for head h land in slot h
        qpT = persist.tile([128, 2, NQ], F16, name="qpT")
        cpT = persist.tile([128, 2, NQ], F16, name="cpT")
        k1hT = persist.tile([128, 2, NK], F16, name="k1hT")
        k2hT = persist.tile([128, 2, NK], F16, name="k2hT")
        vh = persist.tile([128, 32, 2, 128], F16, name="vh")
        u_store = persist.tile([128, 2, 16, 128], F16, name="u_store")
        # 1/r per (head, tile, m): rinv_store[:, h, t, 0] = 1/r1 feeds the
        # deferred out-proj normalization; [..., 1] = 1/r2 feeds the combine.
        rinv_store = persist.tile([128, 2, 16, 2], F32, name="rinv_store")

        acts = ctx.enter_context(tc.tile_pool(name="acts", bufs=2))
        ppsum = ctx.enter_context(tc.tile_pool(name="ppsum", bufs=2, space="PSUM"))
        spsum = ctx.enter_context(tc.tile_pool(name="spsum", bufs=2, space="PSUM"))
        ptpsum = ctx.enter_context(tc.tile_pool(name="ptpsum", bufs=1, space="PSUM"))
        upsum = ctx.enter_context(tc.tile_pool(name="upsum", bufs=1, space="PSUM"))
        ppool = ctx.enter_context(tc.tile_pool(name="ppool", bufs=4))
        ptpool = ctx.enter_context(tc.tile_pool(name="ptpool", bufs=3))
        small = ctx.enter_context(tc.tile_pool(name="small", bufs=4))
        opool = ctx.enter_context(tc.tile_pool(name="opool", bufs=2))

        def proj_one(src_, wname, bname, dst, blk):
            a_sb = acts.tile([128, 8, 512], F16, tag="act_in")
            nc.sync.dma_start(
                out=a_sb,
                in_=src_.rearrange("(c p) n -> p c n", p=128)[
                    :, :, blk * 512 : (blk + 1) * 512
                ],
            )
            for h in range(2):
                hs = slice(h * 128, (h + 1) * 128)
                ps = ppsum.tile([128, 512], F32, tag="proj")
                for e in range(8):
                    nc.tensor.matmul(
                        ps,
                        lhsT=w_sb[wname][:, e, hs],
                        rhs=a_sb[:, e, :],
                        start=(e == 0),
                        stop=(e == 7),
                    )
                nc.vector.tensor_scalar_add(
                    dst[:, h, blk * 512 : (blk + 1) * 512],
                    ps,
                    b_sb[bname][:, h : h + 1],
                )

        def proj_qc_block(blk):
            proj_one(qT, "wq", "bq", qpT, blk)
            proj_one(cT, "wc", "bc", cpT, blk)

        def proj_k_block(blk):
            a_sb = acts.tile([128, 8, 512], F16, tag="act_in")
            nc.sync.dma_start(
                out=a_sb,
                in_=kT.rearrange("(c p) n -> p c n", p=128)[
                    :, :, blk * 512 : (blk + 1) * 512
                ],
            )
            for wname, bname, dst in (("wk1", "bk1", k1hT), ("wk2", "bk2", k2hT)):
                for h in range(2):
                    hs = slice(h * 128, (h + 1) * 128)
                    ps = ppsum.tile([128, 512], F32, tag="proj")
                    for e in range(8):
                        nc.tensor.matmul(
                            ps,
                            lhsT=w_sb[wname][:, e, hs],
                            rhs=a_sb[:, e, :],
                            start=(e == 0),
                            stop=(e == 7),
                        )
                    nc.vector.tensor_scalar_add(
                        dst[:, h, blk * 512 : (blk + 1) * 512],
                        ps,
                        b_sb[bname][:, h : h + 1],
                    )

        def proj_v_block(blk):
            a_sb = acts.tile([128, 8, 512], F16, tag="act_in")
            nc.sync.dma_start(
                out=a_sb,
                in_=vT.rearrange("(c p) n -> p c n", p=128)[
                    :, :, blk * 512 : (blk + 1) * 512
                ],
            )
            for tp in range(2):
                ps = ppsum.tile([128, 512], F32, tag="proj", name="vps")
                for half in range(2):
                    tt = tp * 2 + half
                    for e in range(8):
                        nc.tensor.matmul(
                            ps[:, half * 256 : (half + 1) * 256],
                            lhsT=a_sb[:, e, tt * 128 : (tt + 1) * 128],
                            rhs=w_sb["wv"][:, e, :],
                            start=(e == 0),
                            stop=(e == 7),
                        )
                nc.vector.tensor_copy(
                    out=vh[:, blk * 4 + tp * 2 : blk * 4 + tp * 2 + 2, :, :], in_=ps
                )

        def attn_scores_q(t, h, m, qt, state):
            """Scores+exp for one quarter of one score matrix m of tile t."""
            src_, khT = (qpT, k1hT) if m == 1 else (cpT, k2hT)
            if qt == 0:
                if "rp" not in state:
                    state["rp"] = small.tile(
                        [128, 2, 4], F32, tag="rp", name=f"rp_{t}_{h}"
                    )
                state[m] = ppool.tile(
                    [128, NK],
                    F16,
                    tag=f"p{m}",
                    bufs=5,
                    name=f"p{m}_{t}_{h}",
                )
            p_sb = state[m]
            rp_m = state["rp"][:, m - 1, :]
            lhsT = src_[:, h, t * 128 : (t + 1) * 128]
            ps = spsum.tile([128, 1024], F32, tag="s", name=f"s{m}_{t}_{h}_{qt}")
            for j in range(2):
                nc.tensor.matmul(
                    ps[:, j * 512 : (j + 1) * 512],
                    lhsT=lhsT,
                    rhs=khT[:, h, qt * 1024 + j * 512 : qt * 1024 + (j + 1) * 512],
                    start=True,
                    stop=True,
                )
            nc.scalar.activation(
                out=p_sb[:, qt * 1024 : (qt + 1) * 1024],
                in_=ps,
                func=ACTF.Exp,
                scale=SCALE,
                accum_out=rp_m[:, qt : qt + 1],
            )

        def tail_start(t, h, state):
            """Row-sum reduce, 1/r, and the fused softmax combine (DVE)."""
            pc = state[1]
            p2 = state[2]
            r = small.tile([128, 2], F32, tag="r2")
            nc.vector.tensor_reduce(out=r, in_=state["rp"], axis=AX.X, op=ALU.add)
            rinv = rinv_store[:, h, t, :]
            nc.vector.reciprocal(out=rinv, in_=r)
            c21 = small.tile([128, 1], F32, tag="c21")
            nc.vector.tensor_mul(out=c21, in0=r[:, 0:1], in1=rinv[:, 1:2])
            for half in range(2):
                sl = slice(half * 2048, (half + 1) * 2048)
                nc.vector.scalar_tensor_tensor(
                    out=pc[:, sl],
                    in0=p2[:, sl],
                    scalar=c21,
                    in1=pc[:, sl],
                    op0=ALU.mult,
                    op1=ALU.add,
                )
            state["u"] = upsum.tile([128, 128], F32, tag="u", name=f"u_{t}_{h}")

        def tail_T(t, h, state, qt):
            """Transpose one quarter of the combined p and copy to SBUF."""
            pc = state[1]
            pt_ps = ptpsum.tile([128, 1024], F16, tag="pt")
            for c8 in range(8):
                ck = qt * 8 + c8
                nc.tensor.transpose(
                    pt_ps[:, c8 * 128 : (c8 + 1) * 128],
                    in_=pc[:, ck * 128 : (ck + 1) * 128],
                    identity=identity16,
                )
            pt_sb = ptpool.tile([128, 1024], F16, tag="pt_sb", name=f"pt_{t}_{h}_{qt}")
            nc.vector.tensor_copy(out=pt_sb, in_=pt_ps)
            state[f"pt{qt}"] = pt_sb

        def tail_P(t, h, state, qt):
            """attn @ v for one quarter of the keys (accumulates into u)."""
            pt_sb = state[f"pt{qt}"]
            for c8 in range(8):
                ck = qt * 8 + c8
                nc.tensor.matmul(
                    state["u"],
                    lhsT=vh[:, ck, h, :],
                    rhs=pt_sb[:, c8 * 128 : (c8 + 1) * 128],
                    start=(ck == 0),
                    stop=(ck == 31),
                )

        def tail_end(t, h, state):
            nc.vector.tensor_copy(out=u_store[:, h, t, :], in_=state["u"])
            if h == 1:
                o_sb = opool.tile([128, E], F16, tag="o_sb")
                for j in range(2):
                    o_ps0 = ppsum.tile([128, 512], F32, tag="proj", name="ops0")
                    nc.tensor.matmul(
                        o_ps0,
                        lhsT=u_store[:, 0, t, :],
                        rhs=wo_sb[:, 0, j * 512 : (j + 1) * 512],
                        start=True,
                        stop=True,
                    )
                    o_ps1 = ppsum.tile([128, 512], F32, tag="proj", name="ops1")
                    nc.tensor.matmul(
                        o_ps1,
                        lhsT=u_store[:, 1, t, :],
                        rhs=wo_sb[:, 1, j * 512 : (j + 1) * 512],
                        start=True,
                        stop=True,
                    )
                    otmp = opool.tile([128, 512], F16, tag="otmp")
                    nc.vector.tensor_scalar_mul(
                        otmp, o_ps1, rinv_store[:, 1, t, 0:1]
                    )
                    nc.vector.scalar_tensor_tensor(
                        out=o_sb[:, j * 512 : (j + 1) * 512],
                        in0=o_ps0,
                        scalar=rinv_store[:, 0, t, 0:1],
                        in1=otmp,
                        op0=ALU.mult,
                        op1=ALU.add,
                    )
                nc.sync.dma_start(out=outp[t * 128 : (t + 1) * 128, :], in_=o_sb)

        def tail_pieces(t, h, state):
            """7 tail pieces for chunk-level interleaving.  The DVE combine
            gets its own piece so the PE FIFO is not queued right behind it."""
            yield lambda: tail_start(t, h, state)
            yield lambda: tail_T(t, h, state, 0)
            yield lambda: (tail_T(t, h, state, 1), tail_P(t, h, state, 0))
            yield lambda: (tail_T(t, h, state, 2), tail_P(t, h, state, 1))
            yield lambda: (tail_T(t, h, state, 3), tail_P(t, h, state, 2))
            yield lambda: tail_P(t, h, state, 3)
            yield lambda: tail_end(t, h, state)

        # ---- phase A: q/c block 0 (DMA-ordered), k blocks with streamed
        # scores+exp for tiles 0..3 of head 0.
        _load_w("wq")
        _load_b("bq")
        proj_one(qT, "wq", "bq", qpT, 0)
        _load_w("wc")
        _load_b("bc")
        proj_one(cT, "wc", "bc", cpT, 0)
        _load_w("wk1")
        _load_w("wk2")
        _load_b("bk1")
        _load_b("bk2")
        states = {}
        for t in range(4):
            states[(t, 0)] = {}
        for blk in range(8):
            proj_k_block(blk)
            if blk % 2 == 1:
                qt = blk // 2
                for t in range(4):
                    for m in (1, 2):
                        attn_scores_q(t, 0, m, qt, states[(t, 0)])
        _load_w("wv")
        identity16 = const.tile([128, 128], F16)
        make_identity(nc, identity16)
        wo_sb = const.tile([128, 2, E], F16)
        nc.sync.dma_start(out=wo_sb, in_=wo.rearrange("(c p) n -> p c n", p=128))

        # ---- slot-based weave.  Slot k scores tile S[k] (8 chunks) and runs
        # the tail of TAILS[k-4] (7 pieces), chunk-interleaved, so every
        # engine FIFO always has ready work and tails never overtake their
        # scores in program order.  The v/qc projection blocks are front-
        # loaded extra PE work for the first slots (tails t0..3 P-pieces need
        # vh complete, so tails start at slot 4, after v7 lands in slot 3).
        S = [(t, 0) for t in range(4, 16)] + [(t, 1) for t in range(16)]
        for th in S:
            states[th] = {}
        TAILS = [(t, 0) for t in range(4)] + S

        EXTRA = [
            [
                lambda: proj_one(qT, "wq", "bq", qpT, 1),
                lambda: proj_one(cT, "wc", "bc", cpT, 1),
                lambda: proj_v_block(0),
            ],
            [
                lambda: proj_v_block(1),
                lambda: proj_v_block(2),
                lambda: proj_v_block(3),
            ],
            [
                lambda: proj_v_block(4),
                lambda: proj_v_block(5),
                lambda: proj_one(qT, "wq", "bq", qpT, 2),
            ],
            [
                lambda: proj_v_block(6),
                lambda: proj_v_block(7),
                lambda: proj_one(cT, "wc", "bc", cpT, 2),
            ],
            [lambda: proj_one(qT, "wq", "bq", qpT, 3)],
            [lambda: proj_one(cT, "wc", "bc", cpT, 3)],
        ]

        for k in range(len(TAILS) + 4):
            ex = EXTRA[k] if k < len(EXTRA) else []
            if k < len(S):
                t, h = S[k]
                st = states[(t, h)]
                sc = [
                    (lambda t=t, h=h, m=m, qt=qt, st=st: attn_scores_q(t, h, m, qt, st))
                    for m in (1, 2)
                    for qt in range(4)
                ]
            else:
                sc = []
            tl = list(tail_pieces(*TAILS[k - 4], states[TAILS[k - 4]])) if k >= 4 else []
            for i in range(8):
                if i < len(ex):
                    ex[i]()
                if i < len(sc):
                    sc[i]()
                if i < len(tl):
                    tl[i]()

    nc.compile()
    return nc


def _get_nc():
    global _CACHED_NC
    if _CACHED_NC is None:
        _CACHED_NC = _build()
    return _CACHED_NC


def make_in_maps(q, k, v, cond_feat, Wq, Wc, Wk2, Wv, Wo, bq, bc, bk2):
    f16 = lambda x: np.ascontiguousarray(x, dtype=np.float16)
    b2 = lambda x: np.ascontiguousarray(x.reshape(2, 128).T, dtype=np.float32)

    in_maps = []
    for core in range(8):
        b, g = core // 4, core % 4
        sl = slice(g * 256, (g + 1) * 256)
        sl2 = slice(E + g * 256, E + (g + 1) * 256)
        in_maps.append(
            {
                "qt": f16(q[b].T),
                "ct": f16(cond_feat[b].T),
                "kt": f16(k[b].T),
                "vt": f16(v[b].T),
                "wq": f16(Wq[:, sl]),
                "wc": f16(Wc[:, sl]),
                "wk1": f16(Wk2[:, sl]),
                "wk2": f16(Wk2[:, sl2]),
                "wv": f16(Wv[:, sl]),
                "wo": f16(Wo[sl, :] * 0.5),
                "bq": b2(bq[sl]),
                "bc": b2(bc[sl]),
                "bk1": b2(bk2[sl]),
                "bk2": b2(bk2[sl2]),
            }
        )
    return in_maps


def kernel(q, k, v, cond_feat, Wq, bq, Wc, bc, Wk2, bk2, Wv, bv, Wo, bo):
    from concourse.bass_utils import run_bass_kernel_spmd

    q = np.asarray(q, np.float32)
    k = np.asarray(k, np.float32)
    v = np.asarray(v, np.float32)
    cond_feat = np.asarray(cond_feat, np.float32)
    Wq, bq = np.asarray(Wq, np.float32), np.asarray(bq, np.float32)
    Wc, bc = np.asarray(Wc, np.float32), np.asarray(bc, np.float32)
    Wk2, bk2 = np.asarray(Wk2, np.float32), np.asarray(bk2, np.float32)
    Wv, bv = np.asarray(Wv, np.float32), np.asarray(bv, np.float32)
    Wo, bo = np.asarray(Wo, np.float32), np.asarray(bo, np.float32)

    in_maps = make_in_maps(q, k, v, cond_feat, Wq, Wc, Wk2, Wv, Wo, bq, bc, bk2)

    nc = _get_nc()
    res = run_bass_kernel_spmd(nc, in_maps, core_ids=list(range(8)))

    out = np.zeros((2, NQ, E), np.float32)
    for core in range(8):
        out[core // 4] += res.results[core]["outp"].astype(np.float32)
    out += (bv @ Wo + bo)[None, None, :]
    return out
